# revision 1
# baseline (speedup 1.0000x reference)
"""Bass/Trainium2 kernel for a 2-layer bidirectional QRNN (fo-pooling).

Reference computation (per layer, per direction):
    ZFO = X @ W + b            # [S, B, 3H]
    Z, F, O = split(ZFO); Z = tanh(Z); F = sigmoid(F); O = sigmoid(O)
    c_t = F_t * c_{t-1} + (1 - F_t) * Z_t        (bw direction: reversed time)
    Y_dir = O * C
    Y = concat(Y_fw, Y_bw)     # [S, B, 2H]
Two stacked layers; output is [B, S, 2H].

Sharding: data-parallel over batch. B=16 rows -> 2 rows per NeuronCore x 8.
Each core runs both layers for its 2 rows; no collectives.

Device layout: everything is feature-major ([feat, seq] per batch row) so the
matmul (which contracts over the partition axis) needs no on-chip transposes:
layer-0 input is host-pre-transposed X^T, layer-0 output Y1 is produced
feature-major (exactly what layer 1 consumes via a DRAM round-trip), and the
final output is un-transposed on the host.

The time recurrence uses the DVE `tensor_tensor_scan` instruction
(state = f*state + g along the free axis); the bw direction runs the scan
through reversed access patterns with s-tiles processed in descending order,
chaining the carry via a [128,1] column copy.

mm_dtype="fp32r" (default) computes the gate projections in fp32r (TF32-like
10-bit-mantissa rounding, 4x the fp32 PE rate; measured end-to-end relative
error ~3e-4). fp32r operand tiles must be produced by a compute-engine cast:
a fp32r DMA faults the exec unit and a plain bitcast fails BIR verification.
mm_dtype="fp32" is the exact-precision fallback (~3x slower).
"""

import numpy as np

import concourse.bacc as bacc
import concourse.mybir as mybir
from concourse import bass_utils
from concourse.tile import TileContext

# problem dims (hardcoded per spec)
B, S, D, H = 16, 2048, 512, 512
N_CORES = 8
BC = B // N_CORES  # batch rows per core
P = 128  # SBUF partitions
S_TILE = 512

F32 = mybir.dt.float32
ACT = mybir.ActivationFunctionType
ALU = mybir.AluOpType


def build_nc(bc=BC, s=S, d=D, h=H, s_tile=S_TILE, mm_dtype="fp32r"):
    """Build the SPMD Bass program (same program on every core)."""
    nc = bacc.Bacc("TRN2", target_bir_lowering=False)

    xt = nc.dram_tensor("xt", [bc, d, s], F32, kind="ExternalInput")
    w0f = nc.dram_tensor("w0f", [d, 3 * h], F32, kind="ExternalInput")
    w0b = nc.dram_tensor("w0b", [d, 3 * h], F32, kind="ExternalInput")
    b0f = nc.dram_tensor("b0f", [3 * h], F32, kind="ExternalInput")
    b0b = nc.dram_tensor("b0b", [3 * h], F32, kind="ExternalInput")
    w1f = nc.dram_tensor("w1f", [2 * h, 3 * h], F32, kind="ExternalInput")
    w1b = nc.dram_tensor("w1b", [2 * h, 3 * h], F32, kind="ExternalInput")
    b1f = nc.dram_tensor("b1f", [3 * h], F32, kind="ExternalInput")
    b1b = nc.dram_tensor("b1b", [3 * h], F32, kind="ExternalInput")
    y1 = nc.dram_tensor("y1", [bc, 2 * h, s], F32)  # layer-0 out / layer-1 in
    out_t = nc.dram_tensor("out_t", [bc, 2 * h, s], F32, kind="ExternalOutput")

    ns = s // s_tile
    hc = h // P
    mmdt = mybir.dt.float32r if mm_dtype == "fp32r" else F32

    # DMA queue split: input streams and output writes ride the sync HWDGE
    # queue; weights and biases ride the scalar HWDGE queue (the only two HW
    # DGE queues). A dma_start costs ~600ns on the ISSUING engine, and every
    # engine executes its stream in order — so bulk weight loads are broken
    # into small thunks (one DMA issue or one cast each) and drip-fed through
    # the preceding pass's iterations, where they fit into engine slack.
    STAGE_BUFS = 4

    def weight_load_items(pool, stage_pool, wd, k_chunks, prefix):
        """Create the [P, 3h] weight tiles for one (layer, direction) and
        return (tiles, items): items are thunks (DMA issues into shared
        staging slots, interleaved with ScalarE fp32r casts; BIR verification
        requires the matmul operand's producer to be a rounding compute op).
        Emitting an item never blocks — staging-slot recycling only
        back-pressures the scalar DMA queue at runtime."""
        tiles = [
            pool.tile([P, 3 * h], mmdt, tag=f"{prefix}_wk{k}", name=f"{prefix}_wk{k}")
            for k in range(k_chunks)
        ]
        if mmdt is F32:
            items = [
                (lambda k=k: nc.scalar.dma_start(tiles[k][:], wd[k * P : (k + 1) * P, :]))
                for k in range(k_chunks)
            ]
            return tiles, items
        stgs = {}

        def dma_item(k):
            stg = stage_pool.tile([P, 3 * h], F32, tag="wstg", bufs=STAGE_BUFS,
                                  name=f"{prefix}_stg{k}")
            stgs[k] = stg
            nc.scalar.dma_start(stg[:], wd[k * P : (k + 1) * P, :])

        def cast_item(k):
            nc.scalar.copy(tiles[k][:], stgs.pop(k)[:])

        dmas = [(lambda k=k: dma_item(k)) for k in range(k_chunks)]
        casts = [(lambda k=k: cast_item(k)) for k in range(k_chunks)]
        # interleave with a STAGE_BUFS lead so a cast's DMA is long issued
        items = dmas[:STAGE_BUFS]
        for i in range(k_chunks):
            items.append(casts[i])
            if STAGE_BUFS + i < k_chunks:
                items.append(dmas[STAGE_BUFS + i])
        return tiles, items

    def load_biases(pool, bd, prefix):
        """One DMA loads the whole [3h] bias vector as a [P, 3*hc] column
        table; returns per-(gate, h-chunk) [P, 1] views."""
        btab = pool.tile([P, 3 * hc], F32, tag=f"{prefix}_btab", name=f"{prefix}_btab")
        nc.scalar.dma_start(btab[:], bd[:].rearrange("(j p) -> p j", p=P))
        return {
            (g, hh): btab[:, g * hc + hh : g * hc + hh + 1]
            for g in range(3)
            for hh in range(hc)
        }

    def direction_pass(pools, layer, fw, src, wk, btile, dst, drip=()):
        """One (layer, direction) pass over all batch rows.

        src: DRAM input [bc, Din, s] (xt for layer 0, y1 for layer 1).
        dst: DRAM output [bc, 2h, s]; writes rows [dir_off, dir_off + h).
        drip: deferred small thunks (next passes' weight casts), emitted one
              per iteration so they slot into engine-stream slack.
        """
        spool, cpool, ypool, ppool = pools
        drip_iter = iter(drip)
        n_iters = bc * ns
        per_drip = -(-len(drip) // max(n_iters - 1, 1)) if drip else 0
        k_chunks = (d if layer == 0 else 2 * h) // P
        dir_off = 0 if fw else h
        s_order = list(range(ns)) if fw else list(range(ns - 1, -1, -1))
        for b in range(bc):
            carry = [cpool.tile([P, 1], F32, tag=f"c{hh}", name=f"carry{hh}") for hh in range(hc)]
            for si, s_idx in enumerate(s_order):
                s0 = s_idx * s_tile
                ins = []
                for k in range(k_chunks):
                    if mmdt is F32:
                        t = ypool.tile([P, s_tile], F32, tag=f"inr{k}", name=f"in{k}")
                        nc.sync.dma_start(
                            t[:], src[b, k * P : (k + 1) * P, s0 : s0 + s_tile]
                        )
                    else:
                        stg = ypool.tile([P, s_tile], F32, tag="instg", bufs=4, name="instg")
                        nc.sync.dma_start(
                            stg[:], src[b, k * P : (k + 1) * P, s0 : s0 + s_tile]
                        )
                        t = ypool.tile([P, s_tile], mmdt, tag=f"inr{k}", name=f"inr{k}")
                        nc.vector.tensor_copy(t[:], stg[:])
                    ins.append(t[:])
                for hh in range(hc):
                    ps = [
                        ppool.tile([P, s_tile], F32, tag=f"ps{g}", name=f"ps{g}",
                                   bufs=(3 if g < 2 else 2))
                        for g in range(3)
                    ]
                    for g in range(3):
                        cols = slice(g * h + hh * P, g * h + (hh + 1) * P)
                        for k in range(k_chunks):
                            nc.tensor.matmul(
                                ps[g][:],
                                wk[k][:, cols],
                                ins[k],
                                start=(k == 0),
                                stop=(k == k_chunks - 1),
                            )
                    z = spool.tile([P, s_tile], F32, tag="z", name="z")
                    f_ = spool.tile([P, s_tile], F32, tag="f", name="f")
                    o = spool.tile([P, s_tile], F32, tag="o", name="o")
                    fn = spool.tile([P, s_tile], F32, tag="fn", name="fn")
                    g_ = spool.tile([P, s_tile], F32, tag="g", name="g")
                    c = spool.tile([P, s_tile], F32, tag="c", name="c")
                    y = spool.tile([P, s_tile], F32, tag="y", name="y")
                    nc.scalar.activation(z[:], ps[0][:], ACT.Tanh, bias=btile[0, hh][:])
                    nc.scalar.activation(f_[:], ps[1][:], ACT.Sigmoid, bias=btile[1, hh][:])
                    nc.scalar.activation(o[:], ps[2][:], ACT.Sigmoid, bias=btile[2, hh][:])
                    # g = (1 - f) * z   (1-f on the otherwise idle GPSIMD)
                    nc.gpsimd.tensor_scalar(fn[:], f_[:], -1.0, 1.0, ALU.mult, ALU.add)
                    nc.vector.tensor_mul(g_[:], fn[:], z[:])
                    # c_t = f_t * c_prev + g_t (bw: time runs backwards)
                    if fw:
                        sc = (c[:], f_[:], g_[:])
                        carry_col = slice(s_tile - 1, s_tile)
                    else:
                        sc = (c[:, ::-1], f_[:, ::-1], g_[:, ::-1])
                        carry_col = slice(0, 1)
                    init = 0.0 if si == 0 else carry[hh][:]
                    nc.vector.tensor_tensor_scan(
                        sc[0], sc[1], sc[2], init, ALU.mult, ALU.add
                    )
                    if si < ns - 1:
                        nc.gpsimd.tensor_copy(carry[hh][:], c[:, carry_col])
                    nc.gpsimd.tensor_mul(y[:], o[:], c[:])
                    row0 = dir_off + hh * P
                    nc.sync.dma_start(dst[b, row0 : row0 + P, s0 : s0 + s_tile], y[:])
                if b > 0 or si > 0:
                    for _ in range(per_drip):
                        thunk = next(drip_iter, None)
                        if thunk is not None:
                            thunk()
        for thunk in drip_iter:
            thunk()

    with TileContext(nc) as tc:
        # One shared set of working pools across all four passes -- pool
        # boundaries create SBUF-address-reuse barriers (the next pass's
        # first tiles wait for the old pool's last readers), while shared
        # tags hand off slot-by-slot and keep the PE fed.
        with (
            tc.tile_pool(name="shared", bufs=1) as shpool,
            tc.tile_pool(name="scr", bufs=3) as spool,
            tc.tile_pool(name="carry", bufs=1) as cpool,
            tc.tile_pool(name="instream", bufs=2) as ypool,
            tc.tile_pool(name="ps", bufs=2, space="PSUM") as ppool,
            tc.tile_pool(name="w1f_pool", bufs=1) as w1f_pool,
        ):
            pools = (spool, cpool, ypool, ppool)

            # -------- layer 0 (streams xt, writes y1) --------
            with tc.tile_pool(name="w0_pool", bufs=1) as w0_pool:
                # w0f loads+casts run immediately (they gate the kernel
                # start); everything else is issued up front on the scalar
                # queue but cast via drip.
                wk0f, it0f = weight_load_items(w0_pool, shpool, w0f, d // P, "w0f")
                for it in it0f:
                    it()
                bt0f = load_biases(w0_pool, b0f, "w0f")
                bt0b = load_biases(w0_pool, b0b, "w0b")
                bt1f = load_biases(w1f_pool, b1f, "w1f")
                wk0b, it0b = weight_load_items(w0_pool, shpool, w0b, d // P, "w0b")
                wk1f, it1f = weight_load_items(w1f_pool, shpool, w1f, 2 * h // P, "w1f")
                direction_pass(pools, 0, True, xt, wk0f, bt0f, y1, drip=it0b + it1f[:4])
                direction_pass(pools, 0, False, xt, wk0b, bt0b, y1, drip=it1f[4:])

            # -------- layer 1 (streams y1, writes out_t) --------
            with tc.tile_pool(name="w1b_pool", bufs=1) as w1b_pool:
                bt1b = load_biases(w1b_pool, b1b, "w1b")
                wk1b, it1b = weight_load_items(w1b_pool, shpool, w1b, 2 * h // P, "w1b")
                direction_pass(pools, 1, True, y1, wk1f, bt1f, out_t, drip=it1b)
                direction_pass(pools, 1, False, y1, wk1b, bt1b, out_t)

    nc.finalize()
    return nc


_NC_CACHE = {}


def _get_nc(mm_dtype):
    if mm_dtype not in _NC_CACHE:
        _NC_CACHE[mm_dtype] = build_nc(mm_dtype=mm_dtype)
    return _NC_CACHE[mm_dtype]


def kernel(X, seqlens, W_fw0, b_fw0, W_bw0, b_bw0, W_fw1, b_fw1, W_bw1, b_bw1,
           mm_dtype="fp32r", trace=False):
    """Full-input entry point: shards over 8 cores, returns [B, S, 2H] f32."""
    del seqlens  # unused by the reference computation
    X = np.ascontiguousarray(np.asarray(X, dtype=np.float32))
    weights = {
        "w0f": W_fw0, "b0f": b_fw0, "w0b": W_bw0, "b0b": b_bw0,
        "w1f": W_fw1, "b1f": b_fw1, "w1b": W_bw1, "b1b": b_bw1,
    }
    weights = {k: np.ascontiguousarray(np.asarray(v, dtype=np.float32))
               for k, v in weights.items()}

    nc = _get_nc(mm_dtype)
    in_maps = []
    for i in range(N_CORES):
        rows = X[i * BC : (i + 1) * BC]  # [BC, S, D]
        xt_i = np.ascontiguousarray(rows.transpose(0, 2, 1))  # [BC, D, S]
        in_maps.append({"xt": xt_i, **weights})

    res = bass_utils.run_bass_kernel_spmd(
        nc, in_maps, core_ids=list(range(N_CORES)), trace=trace
    )
    out = np.empty((B, S, 2 * H), dtype=np.float32)
    for i in range(N_CORES):
        out_t = res.results[i]["out_t"]  # [BC, 2H, S]
        out[i * BC : (i + 1) * BC] = out_t.transpose(0, 2, 1)
    kernel.last_results = res
    return out



# revision 2
# speedup vs baseline: 1.1600x; 1.1600x over previous
"""Bass/Trainium2 kernel for a 2-layer bidirectional QRNN (fo-pooling).

Reference computation (per layer, per direction):
    ZFO = X @ W + b            # [S, B, 3H]
    Z, F, O = split(ZFO); Z = tanh(Z); F = sigmoid(F); O = sigmoid(O)
    c_t = F_t * c_{t-1} + (1 - F_t) * Z_t        (bw direction: reversed time)
    Y_dir = O * C
    Y = concat(Y_fw, Y_bw)     # [S, B, 2H]
Two stacked layers; output is [B, S, 2H].

Sharding: data-parallel over batch. B=16 rows -> 2 rows per NeuronCore x 8.
Each core runs both layers for its 2 rows; no collectives.

v2 design (635us -> target ~PE roofline):
- All matmul operands are fp16 (1 cyc/row on the PE, same as fp32r, but
  2-byte LDWEIGHTS and no compute-producer restriction: weights and X are
  host-pre-cast and DMA'd directly; the fp32r staging/cast machinery and the
  192 DVE input casts of v1 are gone). Whole-pipeline fp16 rel-err vs the
  fp32 reference is ~1.4e-3 (CPU-simulated exactly).
- X ([BC,D,S] fp16, 4MB/core) and the layer-0 output y1 ([BC,2H,S] fp16,
  8MB/core) are SBUF-resident; y1 never round-trips DRAM. Total HBM traffic
  drops ~103MB -> ~21MB/core, and DMA issues from 348 -> ~80.
- Gate combine fused: one DVE scalar_tensor_tensor computes g' = (f-1)*z and
  the DVE scan runs state = f*state - g' = f*state + (1-f)*z, so the
  elementwise work per [128,512] tile is 3 scalar activations + 2 DVE ops +
  1 gpsimd mul -- all hidden under the 12 (layer0) / 24 (layer1) matmuls.
- Pass order l0-fw, l0-bw, l1-bw, l1-fw: l1-bw consumes y1 s-tiles in the
  order l0-bw produces them (descending), and l1-fw's first s-tile needs
  l0-bw's last -- so the PE queue never waits at a pass boundary.
"""

import numpy as np

import concourse.bacc as bacc
import concourse.mybir as mybir
from concourse import bass_utils
from concourse.tile import TileContext

# problem dims (hardcoded per spec)
B, S, D, H = 16, 2048, 512, 512
N_CORES = 8
BC = B // N_CORES  # batch rows per core
P = 128            # SBUF partitions
ST = 512           # s-tile (max moving free dim)
NS = S // ST       # 4 s-tiles
HC = H // P        # 4 hidden chunks
K0 = D // P        # 4 contraction chunks, layer 0
K1 = 2 * H // P    # 8 contraction chunks, layer 1

F32 = mybir.dt.float32
F16 = mybir.dt.float16
ACT = mybir.ActivationFunctionType
ALU = mybir.AluOpType


def build_nc():
    """Build the SPMD Bass program (same program on every core)."""
    nc = bacc.Bacc("TRN2", target_bir_lowering=False)

    xt = nc.dram_tensor("xt", [BC, K0, P, S], F16, kind="ExternalInput")
    w0f = nc.dram_tensor("w0f", [K0, P, 3 * H], F16, kind="ExternalInput")
    w0b = nc.dram_tensor("w0b", [K0, P, 3 * H], F16, kind="ExternalInput")
    w1f = nc.dram_tensor("w1f", [K1, P, 3 * H], F16, kind="ExternalInput")
    w1b = nc.dram_tensor("w1b", [K1, P, 3 * H], F16, kind="ExternalInput")
    b0f = nc.dram_tensor("b0f", [3 * H], F32, kind="ExternalInput")
    b0b = nc.dram_tensor("b0b", [3 * H], F32, kind="ExternalInput")
    b1f = nc.dram_tensor("b1f", [3 * H], F32, kind="ExternalInput")
    b1b = nc.dram_tensor("b1b", [3 * H], F32, kind="ExternalInput")
    # out rows indexed (dir*HC + hh)*P + p -> feature dir*H + hh*P + p
    out_t = nc.dram_tensor("out_t", [BC, 2 * HC, P, S], F16, kind="ExternalOutput")

    with TileContext(nc) as tc:
        with (
            tc.tile_pool(name="res", bufs=1) as rpool,
            tc.tile_pool(name="scr", bufs=3) as spool,
            tc.tile_pool(name="carry", bufs=1) as cpool,
            tc.tile_pool(name="ps", bufs=1, space="PSUM") as ppool,
        ):
            # ---- resident loads (issue everything up front, ordered by
            # first use; weights ride the scalar queue, X the sync queue) ----
            def wload(wd, kc, name):
                wt = rpool.tile([P, kc, 3 * H], F16, name=name)
                nc.scalar.dma_start(wt[:], wd[:].rearrange("k p m -> p k m"))
                return wt

            def bload(bd, name):
                bt = rpool.tile([P, 3 * HC], F32, name=name)
                nc.scalar.dma_start(bt[:], bd[:].rearrange("(j p) -> p j", p=P))
                return bt

            wt0f = wload(w0f, K0, "wt0f")
            bt0f = bload(b0f, "bt0f")
            wt0b = wload(w0b, K0, "wt0b")
            bt0b = bload(b0b, "bt0b")
            wt1b = wload(w1b, K1, "wt1b")
            bt1b = bload(b1b, "bt1b")
            wt1f = wload(w1f, K1, "wt1f")
            bt1f = bload(b1f, "bt1f")

            xres = [[rpool.tile([P, S], F16, name=f"x{b}k{k}") for k in range(K0)]
                    for b in range(BC)]
            for b in range(BC):
                for k in range(K0):
                    nc.sync.dma_start(xres[b][k][:], xt[b, k, :, :])

            # y1 resident, one tile per s-tile: [p, b, kk, s] with kk the
            # layer-1 contraction chunk (fw: 0..3, bw: 4..7)
            y1s = [rpool.tile([P, BC, K1, ST], F16, name=f"y1s{si}")
                   for si in range(NS)]

            def direction_pass(layer, fw, wt, bt, kc):
                dir_off = 0 if fw else HC
                s_order = range(NS) if fw else range(NS - 1, -1, -1)
                for b in range(BC):
                    carry = [cpool.tile([P, 1], F16, name=f"cr{b}_{hh}")
                             for hh in range(HC)]
                    for si, s_idx in enumerate(s_order):
                        s0 = s_idx * ST
                        for hh in range(HC):
                            ps = [
                                ppool.tile([P, ST], F32, tag=f"ps{g}",
                                           name=f"ps{g}", bufs=(3 if g < 2 else 2))
                                for g in range(3)
                            ]
                            for g in range(3):
                                cols = slice(g * H + hh * P, g * H + (hh + 1) * P)
                                for k in range(kc):
                                    mov = (xres[b][k][:, s0:s0 + ST] if layer == 0
                                           else y1s[s_idx][:, b, k, :])
                                    nc.tensor.matmul(
                                        ps[g][:], wt[:, k, cols], mov,
                                        start=(k == 0), stop=(k == kc - 1),
                                    )
                            z = spool.tile([P, ST], F16, tag="z", name="z")
                            f_ = spool.tile([P, ST], F16, tag="f", name="f")
                            o = spool.tile([P, ST], F16, tag="o", name="o")
                            g_ = spool.tile([P, ST], F16, tag="g", name="g")
                            c = spool.tile([P, ST], F16, tag="c", name="c")
                            bcol = lambda gi: bt[:, gi * HC + hh : gi * HC + hh + 1]
                            nc.scalar.activation(z[:], ps[0][:], ACT.Tanh, bias=bcol(0))
                            nc.scalar.activation(f_[:], ps[1][:], ACT.Sigmoid, bias=bcol(1))
                            nc.scalar.activation(o[:], ps[2][:], ACT.Sigmoid, bias=bcol(2))
                            # g' = (f-1)*z; scan: c = f*c - g' = f*c + (1-f)*z
                            nc.vector.scalar_tensor_tensor(
                                g_[:], f_[:], 1.0, z[:], ALU.subtract, ALU.mult
                            )
                            if fw:
                                sc = (c[:], f_[:], g_[:])
                                ccol = slice(ST - 1, ST)
                            else:
                                sc = (c[:, ::-1], f_[:, ::-1], g_[:, ::-1])
                                ccol = slice(0, 1)
                            init = 0.0 if si == 0 else carry[hh][:]
                            nc.vector.tensor_tensor_scan(
                                sc[0], sc[1], sc[2], init, ALU.mult, ALU.subtract
                            )
                            if si < NS - 1:
                                nc.gpsimd.tensor_copy(carry[hh][:], c[:, ccol])
                            if layer == 0:
                                nc.gpsimd.tensor_mul(
                                    y1s[s_idx][:, b, dir_off + hh, :], o[:], c[:]
                                )
                            else:
                                y = spool.tile([P, ST], F16, tag="y", name="y")
                                nc.gpsimd.tensor_mul(y[:], o[:], c[:])
                                nc.sync.dma_start(
                                    out_t[b, dir_off + hh, :, s0:s0 + ST], y[:]
                                )

            direction_pass(0, True, wt0f, bt0f, K0)
            direction_pass(0, False, wt0b, bt0b, K0)
            direction_pass(1, False, wt1b, bt1b, K1)
            direction_pass(1, True, wt1f, bt1f, K1)

    nc.finalize()
    return nc


_NC_CACHE = {}


def _get_nc(variant="fp16"):
    if variant not in _NC_CACHE:
        _NC_CACHE[variant] = build_nc()
    return _NC_CACHE[variant]


def kernel(X, seqlens, W_fw0, b_fw0, W_bw0, b_bw0, W_fw1, b_fw1, W_bw1, b_bw1,
           mm_dtype="fp16", trace=False):
    """Full-input entry point: shards over 8 cores, returns [B, S, 2H] f32."""
    del seqlens  # unused by the reference computation
    X = np.asarray(X, dtype=np.float32)

    def wprep(w, kc):  # [Din, 3H] f32 -> [kc, P, 3H] fp16
        return np.ascontiguousarray(
            np.asarray(w, np.float32).reshape(kc, P, 3 * H).astype(np.float16)
        )

    weights = {
        "w0f": wprep(W_fw0, K0), "w0b": wprep(W_bw0, K0),
        "w1f": wprep(W_fw1, K1), "w1b": wprep(W_bw1, K1),
        "b0f": np.ascontiguousarray(np.asarray(b_fw0, np.float32)),
        "b0b": np.ascontiguousarray(np.asarray(b_bw0, np.float32)),
        "b1f": np.ascontiguousarray(np.asarray(b_fw1, np.float32)),
        "b1b": np.ascontiguousarray(np.asarray(b_bw1, np.float32)),
    }

    nc = _get_nc(mm_dtype)
    in_maps = []
    for i in range(N_CORES):
        rows = X[i * BC : (i + 1) * BC]  # [BC, S, D]
        xt_i = np.ascontiguousarray(
            rows.transpose(0, 2, 1).reshape(BC, K0, P, S).astype(np.float16)
        )
        in_maps.append({"xt": xt_i, **weights})

    res = bass_utils.run_bass_kernel_spmd(
        nc, in_maps, core_ids=list(range(N_CORES)), trace=trace
    )
    out = np.empty((B, S, 2 * H), dtype=np.float32)
    for i in range(N_CORES):
        out_t = res.results[i]["out_t"]  # [BC, 2*HC, P, S] fp16
        out[i * BC : (i + 1) * BC] = (
            out_t.reshape(BC, 2 * H, S).transpose(0, 2, 1).astype(np.float32)
        )
    kernel.last_results = res
    return out


# revision 3
# speedup vs baseline: 1.1991x; 1.0336x over previous
"""Bass/Trainium2 kernel for a 2-layer bidirectional QRNN (fo-pooling).

Reference computation (per layer, per direction):
    ZFO = X @ W + b            # [S, B, 3H]
    Z, F, O = split(ZFO); Z = tanh(Z); F = sigmoid(F); O = sigmoid(O)
    c_t = F_t * c_{t-1} + (1 - F_t) * Z_t        (bw direction: reversed time)
    Y_dir = O * C
    Y = concat(Y_fw, Y_bw)     # [S, B, 2H]
Two stacked layers; output is [B, S, 2H].

Sharding: data-parallel over batch. B=16 rows -> 2 rows per NeuronCore x 8.
Each core runs both layers for its 2 rows; no collectives.

v2 design (635us -> target ~PE roofline):
- All matmul operands are fp16 (1 cyc/row on the PE, same as fp32r, but
  2-byte LDWEIGHTS and no compute-producer restriction: weights and X are
  host-pre-cast and DMA'd directly; the fp32r staging/cast machinery and the
  192 DVE input casts of v1 are gone). Whole-pipeline fp16 rel-err vs the
  fp32 reference is ~1.4e-3 (CPU-simulated exactly).
- X ([BC,D,S] fp16, 4MB/core) and the layer-0 output y1 ([BC,2H,S] fp16,
  8MB/core) are SBUF-resident; y1 never round-trips DRAM. Total HBM traffic
  drops ~103MB -> ~21MB/core, and DMA issues from 348 -> ~80.
- Gate combine fused: one DVE scalar_tensor_tensor computes g' = (f-1)*z and
  the DVE scan runs state = f*state - g' = f*state + (1-f)*z, so the
  elementwise work per [128,512] tile is 3 scalar activations + 2 DVE ops +
  1 gpsimd mul -- all hidden under the 12 (layer0) / 24 (layer1) matmuls.
- Pass order l0-fw, l0-bw, l1-bw, l1-fw: l1-bw consumes y1 s-tiles in the
  order l0-bw produces them (descending), and l1-fw's first s-tile needs
  l0-bw's last -- so the PE queue never waits at a pass boundary.
"""

import numpy as np

import concourse.bacc as bacc
import concourse.mybir as mybir
from concourse import bass_utils
from concourse.tile import TileContext

# problem dims (hardcoded per spec)
B, S, D, H = 16, 2048, 512, 512
N_CORES = 8
BC = B // N_CORES  # batch rows per core
P = 128            # SBUF partitions
ST = 512           # s-tile (max moving free dim)
NS = S // ST       # 4 s-tiles
HC = H // P        # 4 hidden chunks
K0 = D // P        # 4 contraction chunks, layer 0
K1 = 2 * H // P    # 8 contraction chunks, layer 1

F32 = mybir.dt.float32
F16 = mybir.dt.float16
ACT = mybir.ActivationFunctionType
ALU = mybir.AluOpType


def build_nc():
    """Build the SPMD Bass program (same program on every core)."""
    nc = bacc.Bacc("TRN2", target_bir_lowering=False)

    xt = nc.dram_tensor("xt", [BC, K0, P, S], F16, kind="ExternalInput")
    w0f = nc.dram_tensor("w0f", [K0, P, 3 * H], F16, kind="ExternalInput")
    w0b = nc.dram_tensor("w0b", [K0, P, 3 * H], F16, kind="ExternalInput")
    w1f = nc.dram_tensor("w1f", [K1, P, 3 * H], F16, kind="ExternalInput")
    w1b = nc.dram_tensor("w1b", [K1, P, 3 * H], F16, kind="ExternalInput")
    b0f = nc.dram_tensor("b0f", [3 * H], F32, kind="ExternalInput")
    b0b = nc.dram_tensor("b0b", [3 * H], F32, kind="ExternalInput")
    b1f = nc.dram_tensor("b1f", [3 * H], F32, kind="ExternalInput")
    b1b = nc.dram_tensor("b1b", [3 * H], F32, kind="ExternalInput")
    # out rows indexed (dir*HC + hh)*P + p -> feature dir*H + hh*P + p
    out_t = nc.dram_tensor("out_t", [BC, 2 * HC, P, S], F16, kind="ExternalOutput")

    with TileContext(nc) as tc:
        with (
            tc.tile_pool(name="res", bufs=1) as rpool,
            tc.tile_pool(name="scr", bufs=3) as spool,
            tc.tile_pool(name="carry", bufs=1) as cpool,
            tc.tile_pool(name="ps", bufs=1, space="PSUM") as ppool,
        ):
            # ---- resident loads: ALL on the sync HWDGE queue (the scalar
            # queue must stay clear -- activations gate PSUM recycling, and a
            # dma_start costs 0.6-4us on the issuing engine). Per-k chunks
            # (contiguous 2D patterns, cheap descriptors) ordered by first
            # use, so the first matmul is gated only by X[b0] + w0f. ----
            def wload(wd, kc, name):
                wt = rpool.tile([P, kc, 3 * H], F16, name=name)
                return wt, [
                    (lambda k=k, wt=wt, wd=wd: nc.sync.dma_start(
                        wt[:, k, :], wd[k, :, :]))
                    for k in range(kc)
                ]

            def bload(bd, name):
                bt = rpool.tile([P, 3 * HC], F32, name=name)
                nc.sync.dma_start(bt[:], bd[:].rearrange("(j p) -> p j", p=P))
                return bt

            xres = [[rpool.tile([P, S], F16, name=f"x{b}k{k}") for k in range(K0)]
                    for b in range(BC)]

            def xload(b):
                for k in range(K0):
                    nc.sync.dma_start(xres[b][k][:], xt[b, k, :, :])

            wt0f, ld0f = wload(w0f, K0, "wt0f")
            wt0b, ld0b = wload(w0b, K0, "wt0b")
            wt1b, ld1b = wload(w1b, K1, "wt1b")
            wt1f, ld1f = wload(w1f, K1, "wt1f")
            xload(0)
            for it in ld0f:
                it()
            bt0f = bload(b0f, "bt0f")
            xload(1)
            for it in ld0b:
                it()
            bt0b = bload(b0b, "bt0b")
            for it in ld1b:
                it()
            bt1b = bload(b1b, "bt1b")
            for it in ld1f:
                it()
            bt1f = bload(b1f, "bt1f")

            # y1 resident, one tile per s-tile: [p, b, kk, s] with kk the
            # layer-1 contraction chunk (fw: 0..3, bw: 4..7)
            y1s = [rpool.tile([P, BC, K1, ST], F16, name=f"y1s{si}")
                   for si in range(NS)]

            def direction_pass(layer, fw, wt, bt, kc):
                dir_off = 0 if fw else HC
                s_order = range(NS) if fw else range(NS - 1, -1, -1)
                for b in range(BC):
                    carry = [cpool.tile([P, 1], F16, name=f"cr{b}_{hh}")
                             for hh in range(HC)]
                    for si, s_idx in enumerate(s_order):
                        s0 = s_idx * ST
                        for hh in range(HC):
                            ps = [
                                ppool.tile([P, ST], F32, tag=f"ps{g}",
                                           name=f"ps{g}", bufs=(3 if g < 2 else 2))
                                for g in range(3)
                            ]
                            for g in range(3):
                                cols = slice(g * H + hh * P, g * H + (hh + 1) * P)
                                for k in range(kc):
                                    mov = (xres[b][k][:, s0:s0 + ST] if layer == 0
                                           else y1s[s_idx][:, b, k, :])
                                    nc.tensor.matmul(
                                        ps[g][:], wt[:, k, cols], mov,
                                        start=(k == 0), stop=(k == kc - 1),
                                    )
                            z = spool.tile([P, ST], F16, tag="z", name="z")
                            f_ = spool.tile([P, ST], F16, tag="f", name="f")
                            o = spool.tile([P, ST], F16, tag="o", name="o")
                            g_ = spool.tile([P, ST], F16, tag="g", name="g")
                            c = spool.tile([P, ST], F16, tag="c", name="c")
                            bcol = lambda gi: bt[:, gi * HC + hh : gi * HC + hh + 1]
                            nc.scalar.activation(z[:], ps[0][:], ACT.Tanh, bias=bcol(0))
                            nc.scalar.activation(f_[:], ps[1][:], ACT.Sigmoid, bias=bcol(1))
                            nc.scalar.activation(o[:], ps[2][:], ACT.Sigmoid, bias=bcol(2))
                            # g' = (f-1)*z; scan: c = f*c - g' = f*c + (1-f)*z
                            nc.vector.scalar_tensor_tensor(
                                g_[:], f_[:], 1.0, z[:], ALU.subtract, ALU.mult
                            )
                            if fw:
                                sc = (c[:], f_[:], g_[:])
                                ccol = slice(ST - 1, ST)
                            else:
                                sc = (c[:, ::-1], f_[:, ::-1], g_[:, ::-1])
                                ccol = slice(0, 1)
                            init = 0.0 if si == 0 else carry[hh][:]
                            nc.vector.tensor_tensor_scan(
                                sc[0], sc[1], sc[2], init, ALU.mult, ALU.subtract
                            )
                            if si < NS - 1:
                                nc.gpsimd.tensor_copy(carry[hh][:], c[:, ccol])
                            if layer == 0:
                                nc.gpsimd.tensor_mul(
                                    y1s[s_idx][:, b, dir_off + hh, :], o[:], c[:]
                                )
                            else:
                                y = spool.tile([P, ST], F16, tag="y", name="y")
                                nc.gpsimd.tensor_mul(y[:], o[:], c[:])
                                nc.sync.dma_start(
                                    out_t[b, dir_off + hh, :, s0:s0 + ST], y[:]
                                )

            direction_pass(0, True, wt0f, bt0f, K0)
            direction_pass(0, False, wt0b, bt0b, K0)
            direction_pass(1, False, wt1b, bt1b, K1)
            direction_pass(1, True, wt1f, bt1f, K1)

    nc.finalize()
    return nc


_NC_CACHE = {}


def _get_nc(variant="fp16"):
    if variant not in _NC_CACHE:
        _NC_CACHE[variant] = build_nc()
    return _NC_CACHE[variant]


def kernel(X, seqlens, W_fw0, b_fw0, W_bw0, b_bw0, W_fw1, b_fw1, W_bw1, b_bw1,
           mm_dtype="fp16", trace=False):
    """Full-input entry point: shards over 8 cores, returns [B, S, 2H] f32."""
    del seqlens  # unused by the reference computation
    X = np.asarray(X, dtype=np.float32)

    def wprep(w, kc):  # [Din, 3H] f32 -> [kc, P, 3H] fp16
        return np.ascontiguousarray(
            np.asarray(w, np.float32).reshape(kc, P, 3 * H).astype(np.float16)
        )

    weights = {
        "w0f": wprep(W_fw0, K0), "w0b": wprep(W_bw0, K0),
        "w1f": wprep(W_fw1, K1), "w1b": wprep(W_bw1, K1),
        "b0f": np.ascontiguousarray(np.asarray(b_fw0, np.float32)),
        "b0b": np.ascontiguousarray(np.asarray(b_bw0, np.float32)),
        "b1f": np.ascontiguousarray(np.asarray(b_fw1, np.float32)),
        "b1b": np.ascontiguousarray(np.asarray(b_bw1, np.float32)),
    }

    nc = _get_nc(mm_dtype)
    in_maps = []
    for i in range(N_CORES):
        rows = X[i * BC : (i + 1) * BC]  # [BC, S, D]
        xt_i = np.ascontiguousarray(
            rows.transpose(0, 2, 1).reshape(BC, K0, P, S).astype(np.float16)
        )
        in_maps.append({"xt": xt_i, **weights})

    res = bass_utils.run_bass_kernel_spmd(
        nc, in_maps, core_ids=list(range(N_CORES)), trace=trace
    )
    out = np.empty((B, S, 2 * H), dtype=np.float32)
    for i in range(N_CORES):
        out_t = res.results[i]["out_t"]  # [BC, 2*HC, P, S] fp16
        out[i * BC : (i + 1) * BC] = (
            out_t.reshape(BC, 2 * H, S).transpose(0, 2, 1).astype(np.float32)
        )
    kernel.last_results = res
    return out
